# revision 2
# baseline (speedup 1.0000x reference)
"""Trainium2 Bass kernel for nn_MemoryBank_51135880626820 (scatter_memory), v3.

Data-parallel over the query batch across 8 NeuronCores: the [32768, 256]
memory bank is replicated per core, each core handles 1024 query rows.

Per-core pipeline (v3):
  - prep: per 1024-row bank chunk: DMA, scalar row-norms (Square+accum) +
    Rsqrt + normalize-cast to bf16, PE transpose to d-major resident bank
    [128, 2, 32768] bf16, DVE 2x-mode PSUM evacuation. norm^2 per row saved
    to DRAM scratch for the exact-refine gather.
  - screen per qtile (128 queries): 32 supers of 1024 m-cols: 2 accumulating
    bf16 matmuls (stationary = transposed queries, reused) -> PSUM f32;
    DVE 2:1 max-pool of adjacent column pairs -> bf16 pooled [128, 16384];
    DVE max8 + find8 per 2048-pooled window -> 64 candidates (value + local
    idx); candidates packed exactly into f32 as  v*0.5 + gidx*2^-26  so the
    top-10 prune (max8 + match_replace + max8) carries indices for free.
  - refine per qtile: one batched indirect DMA gathers both rows of each
    pooled pair (2KB contiguous descriptors via a [16384, 512] view of the
    bank) + their norm^2; exact f32 re-dot via fused tensor_tensor_reduce;
    self-match mask, top-8, unnormalized softmax weights.
  - weighted sum on PE: per candidate a diagonal-stationary f32r matmul
    accumulating w_j * row_j into PSUM; scalar rescales to ||q|| (softmax
    denominator cancels against the renormalize).

self-contained: hardcodes all shapes; builds and caches the Bass program on
first call.
"""

import sys

for _p in ("/opt/trn_rl_repo",):
    if _p not in sys.path:
        sys.path.insert(0, _p)

import numpy as np

import concourse.bass as bass
import concourse.mybir as mybir
import concourse.tile as tile
from concourse.bass import IndirectOffsetOnAxis
from concourse.masks import make_identity

F32 = mybir.dt.float32
F32R = mybir.dt.float32r
BF16 = mybir.dt.bfloat16
F16 = mybir.dt.float16
U16 = mybir.dt.uint16
U32 = mybir.dt.uint32

N_CORES = 8
B = 8192
B_LOC = B // N_CORES        # 1024
M = 32768
D = 256
K = 8
NQT = B_LOC // 128          # 8 query tiles per core
SUP = 1024                  # matmul super (PSUM tile) width in m columns
NSUP = M // SUP             # 32
PW = SUP // 2               # 512 pooled elems per super
NPOOL = M // 2              # 16384 pooled elems per qtile
WIN = 1024                  # pooled window for max8/find8
NW = NPOOL // WIN           # 8 windows
NCAND = NW * 8              # 64 candidates per qtile
LP = 11                     # pruned candidates (pairs) -> 22 gathered rows
NEG = -1.0e30
SELF_MATCH = 0.9999
EPACK = 2.0 ** -26          # pooled-index packing epsilon (on v*0.5)
DEBUG = False               # adds per-qtile intermediate dumps as outputs


# --------------------------------------------------------------------------
# workarounds for this container's walrus build, which rejects more than one
# sync-wait per instruction ("Too many sync wait commands").
# --------------------------------------------------------------------------
def _install_patches():
    import json

    import bass_rust
    import concourse.bass_utils as _bu
    import concourse.bass2jax as _b2j
    import concourse.tile as tile_mod
    from concourse.tile import TileContext

    if getattr(_bu, "_mb_patched", False):
        return

    try:
        ScopedClock = tile_mod.ScopedClock
    except AttributeError:
        ScopedClock = bass_rust.ScopedClock

    def _patched_drain_and_barrier(self, tick_clock, wait_clock):
        nc = self.nc
        drain_inst = nc.sync.drain()
        wait_clock.add_sem_waits(
            drain_inst.ins, ScopedClock({None: tick_clock.global_clock})
        )
        si = drain_inst.ins.sync_info
        waits = list(si.on_wait) if si is not None and si.on_wait else []
        if len(waits) > 1:
            drain_inst.ins.sync_info = bass_rust.SyncInfo(
                on_wait=[waits[0]],
                on_update=list(si.on_update) if si.on_update else [],
            )
            for w in waits[1:]:
                nop = nc.sync.nop(nofuse=True, hint="tail_wait")
                nop.ins.sync_info = bass_rust.SyncInfo(on_wait=[w], on_update=[])
        nc.all_engine_barrier()
        assert self.sems is not None
        popped = nc._tile_sem_poison_stack.pop()
        assert popped is self._sem_poison
        nc.clear_and_free_semaphores(list(self.sems.allocated().values()))
        nc.all_engine_barrier()

    TileContext._drain_and_barrier = _patched_drain_and_barrier

    def dedup_ldweights(m):
        # consecutive Ldweights with identical operands on the PE stream
        # (only Matmult/NoOp between) reload the same stationary; turn the
        # duplicates into NoOps (keeping their sync waits/updates).
        for fn in m.get("functions", []):
            for bb in fn.get("blocks", []):
                last_key = None
                for ins in bb.get("instructions", []):
                    op = ins.get("opcode")
                    if op == "Ldweights":
                        key = json.dumps(
                            [ins.get("ins"), ins.get("perf_mode"),
                             ins.get("is_transpose"), ins.get("tile_position")],
                            sort_keys=True)
                        if key == last_key:
                            ins["opcode"] = "NoOp"
                            ins["ins"] = []
                            ins["outs"] = []
                            ins["text_hint"] = "dedup_ldw"
                        else:
                            last_key = key
                    elif op in ("Matmult", "NoOp"):
                        pass
                    elif ins.get("engine") == "PE":
                        last_key = None
        return m

    import os as _os

    def split_multiwaits(bir_json):
        m = json.loads(bir_json)
        changed = True
        for fn in m.get("functions", []):
            for bb in fn.get("blocks", []):
                insts = bb.get("instructions", [])
                out = []
                for ins in insts:
                    si = ins.get("sync_info") or {}
                    waits = si.get("on_wait") or []
                    if len(waits) > 1:
                        changed = True
                        for kk, w in enumerate(waits[:-1]):
                            out.append({
                                "debug": ins.get("debug", 0),
                                "engine": ins["engine"],
                                "ins": [],
                                "name": f"{ins['name']}-w{kk}",
                                "opcode": "NoOp",
                                "outs": [],
                                "sync_info": {"on_update": [], "on_wait": [w]},
                                "text_hint": "split_wait",
                            })
                        si = dict(si)
                        si["on_wait"] = [waits[-1]]
                        ins = dict(ins)
                        ins["sync_info"] = si
                    out.append(ins)
                bb["instructions"] = out
        return json.dumps(m).encode()

    _orig_compile = _bu.compile_bir_kernel

    def _patched_compile(bir_json, tmpdir, neff_name="file.neff"):
        if isinstance(bir_json, str):
            bir_json = bir_json.encode()
        return _orig_compile(split_multiwaits(bir_json), tmpdir, neff_name)

    _bu.compile_bir_kernel = _patched_compile
    _b2j.compile_bir_kernel = _patched_compile

    _bu._mb_patched = True


# --------------------------------------------------------------------------
# per-core Bass program
# --------------------------------------------------------------------------
def _build():
    nc = bass.Bass("TRN2", target_bir_lowering=False, debug=False)
    q_in = nc.dram_tensor("q", [B_LOC, D], F32, kind="ExternalInput")
    mem_in = nc.dram_tensor("mem", [M, D], F32, kind="ExternalInput")
    out = nc.dram_tensor("out", [B_LOC, D], F32, kind="ExternalOutput")
    if DEBUG:
        dbg_cv = nc.dram_tensor("dbg_cv", [NQT, 128, NCAND], F32, kind="ExternalOutput")
        dbg_ci = nc.dram_tensor("dbg_ci", [NQT, 128, NCAND], U16, kind="ExternalOutput")
        dbg_rows = nc.dram_tensor("dbg_rows", [NQT, 128, 2 * LP], U32, kind="ExternalOutput")
        dbg_cos = nc.dram_tensor("dbg_cos", [NQT, 128, 2 * LP], F32, kind="ExternalOutput")
        dbg_wts = nc.dram_tensor("dbg_wts", [NQT, 128, 2 * LP], F32, kind="ExternalOutput")

    AF = mybir.ActivationFunctionType
    AL = mybir.AluOpType

    with tile.TileContext(nc) as tc, tc.tile_pool(name="res", bufs=1) as res:
        bankT = res.tile([128, 2, M], BF16, tag="bankT")    # d-major bank
        identB = res.tile([128, 128], BF16, tag="identB")
        identF = res.tile([128, 128], F32, tag="identF")
        make_identity(nc, identB[:])
        make_identity(nc, identF[:])
        nqf = res.tile([128, 3, D], F32, tag="nqf")         # exact normalized q (rotating slots)
        qT = res.tile([128, NQT, 2, 128], BF16, tag="qT")   # stationary queries
        qnorm = res.tile([128, NQT], F32, tag="qnorm")

        # ---- prep: bank chunks of 1024 rows ----
        with (
            tc.tile_pool(name="prep", bufs=2) as pp,
            tc.tile_pool(name="ppsum", bufs=2, space="PSUM") as ppsum,
        ):
            for c in range(M // 1024):
                mt = pp.tile([128, 8, D], F32, tag="mt")
                nc.sync.dma_start(
                    mt[:],
                    mem_in[c * 1024:(c + 1) * 1024, :]
                        .rearrange("(g p) d -> p g d", p=128))
                n2 = pp.tile([128, 8], F32, tag="n2")
                sqs = pp.tile([128, D], F32, tag="sqs")
                sqv = pp.tile([128, D], F32, tag="sqv")
                for g in range(4):
                    nc.scalar.activation(sqs[:], mt[:, g, :], AF.Square,
                                         accum_out=n2[:, g:g + 1])
                for g in range(4, 8):
                    nc.vector.scalar_tensor_tensor(
                        out=sqv[:], in0=mt[:, g, :], scalar=1.0,
                        in1=mt[:, g, :], op0=AL.mult, op1=AL.mult,
                        accum_out=n2[:, g:g + 1])
                nrm = pp.tile([128, 8], F32, tag="nrm")
                nc.scalar.activation(nrm[:], n2[:], AF.Sqrt)
                rn = pp.tile([128, 8], F32, tag="rn")
                nc.vector.reciprocal(rn[:], nrm[:])
                nmb = pp.tile([128, 8, D], BF16, tag="nmb")
                nc.gpsimd.tensor_tensor(
                    out=nmb[:], in0=mt[:],
                    in1=rn[:].rearrange("p (g o) -> p g o", o=1)
                        .to_broadcast([128, 8, D]),
                    op=AL.mult)
                for h in range(2):
                    pt = ppsum.tile([128, 1024], BF16, tag="pt")
                    for g in range(8):
                        nc.tensor.transpose(
                            pt[:, g * 128:(g + 1) * 128],
                            nmb[:, g, h * 128:(h + 1) * 128], identB[:])
                    nc.vector.tensor_copy(
                        bankT[:, h, c * 1024:(c + 1) * 1024], pt[:])

        # ---- main: screen(qt)/refine(qt-1) software-pipelined ----
        with (
            tc.tile_pool(name="wk", bufs=2) as wk,
            tc.tile_pool(name="scrp", bufs=1) as scrp,
            tc.tile_pool(name="pool2", bufs=2) as pool2,
            tc.tile_pool(name="gat", bufs=2) as gat,
            tc.tile_pool(name="gbp", bufs=1) as gbp,
            tc.tile_pool(name="dgp", bufs=2) as dgp,
            tc.tile_pool(name="psS", bufs=3, space="PSUM") as psS,
            tc.tile_pool(name="psQ", bufs=1, space="PSUM") as psQ,
            tc.tile_pool(name="psA", bufs=1, space="PSUM") as psA,
        ):
            mem_pairs = mem_in[:].rearrange("(P two) d -> P (two d)", two=2)

            def screen(qt, mid_cb=None):
                """matmul + pool + window top8 + prune + gather issue.
                Returns state needed by refine(qt)."""
                # -- query prep --
                qf = wk.tile([128, D], F32, tag="qf")
                nc.sync.dma_start(qf[:], q_in[qt * 128:(qt + 1) * 128, :])
                qn2 = wk.tile([128, 1], F32, tag="qn2")
                qsq = scrp.tile([128, D], F32, tag="qsq")
                nc.scalar.activation(qsq[:], qf[:], AF.Square,
                                     accum_out=qn2[:])
                nc.scalar.activation(qnorm[:, qt:qt + 1], qn2[:], AF.Sqrt)
                qr = wk.tile([128, 1], F32, tag="qr")
                nc.vector.reciprocal(qr[:], qnorm[:, qt:qt + 1])
                nc.scalar.activation(nqf[:, qt % 3, :], qf[:], AF.Copy,
                                     scale=qr[:])
                nqb = wk.tile([128, D], BF16, tag="nqb")
                nc.vector.tensor_copy(nqb[:], nqf[:, qt % 3, :])
                pq = psQ.tile([128, 256], BF16, tag="pq")
                for h in range(2):
                    nc.tensor.transpose(pq[:, h * 128:(h + 1) * 128],
                                        nqb[:, h * 128:(h + 1) * 128],
                                        identB[:])
                nc.vector.tensor_copy(
                    qT[:, qt, :, :],
                    pq[:].rearrange("p (h j) -> p h j", h=2))

                # -- matmul + pool + window screen --
                cv = wk.tile([128, NCAND], F16, tag="cv")
                ci = wk.tile([128, NCAND], U16, tag="ci")
                for w in range(NW):
                    if w == 4 and mid_cb is not None:
                        mid_cb()
                    pl = pool2.tile([128, WIN], F16, tag="pl")
                    ps0 = psS.tile([128, SUP], F32, tag="ps")
                    ps1 = psS.tile([128, SUP], F32, tag="ps")
                    pst = [ps0, ps1]
                    # h-outer over a super pair: one stationary load
                    # covers 4 consecutive matmuls (deduped in BIR).
                    for h in range(2):
                        for si in range(2):
                            s = w * 2 + si
                            for b2 in range(2):
                                nc.tensor.matmul(
                                    pst[si][:, b2 * 512:(b2 + 1) * 512],
                                    qT[:, qt, h, :],
                                    bankT[:, h, s * SUP + b2 * 512:
                                          s * SUP + (b2 + 1) * 512],
                                    start=(h == 0), stop=(h == 1))
                    # DVE TT may read at most one PSUM operand: scalar
                    # evacuates the odd elements, DVE maxes vs the evens.
                    for si in range(2):
                        psv = pst[si][:].rearrange(
                            "p (m two) -> p two m", two=2)
                        hb = wk.tile([128, PW], F16, tag="hb")
                        nc.scalar.activation(hb[:], psv[:, 1, :], AF.Copy)
                        nc.vector.tensor_tensor(
                            out=pl[:, si * PW:(si + 1) * PW],
                            in0=psv[:, 0, :], in1=hb[:], op=AL.max)
                    nc.vector.max(out=cv[:, w * 8:(w + 1) * 8], in_=pl[:])
                    nc.vector.max_index(
                        out=ci[:, w * 8:(w + 1) * 8],
                        in_max=cv[:, w * 8:(w + 1) * 8], in_values=pl[:])

                # -- prune to LP: pack (value*0.5 + local_idx*2^-26) --
                cif = wk.tile([128, NCAND], F32, tag="cif")
                nc.vector.tensor_copy(cif[:], ci[:])
                packed = wk.tile([128, NCAND], F32, tag="packed")
                nc.vector.tensor_scalar(out=packed[:], in0=cif[:],
                                        scalar1=EPACK, scalar2=None,
                                        op0=AL.mult)
                hcv = wk.tile([128, NCAND], F32, tag="hcv")
                nc.vector.tensor_scalar(out=hcv[:], in0=cv[:], scalar1=0.5,
                                        scalar2=None, op0=AL.mult)
                nc.vector.tensor_tensor(out=packed[:], in0=packed[:],
                                        in1=hcv[:], op=AL.add)
                pk8 = wk.tile([128, 8], F32, tag="pk8")
                nc.vector.max(out=pk8[:], in_=packed[:])
                pB = wk.tile([128, NCAND], F32, tag="pB")
                nc.vector.match_replace(out=pB[:], in_to_replace=pk8[:],
                                        in_values=packed[:], imm_value=NEG)
                pk2 = wk.tile([128, 8], F32, tag="pk2")
                nc.vector.max(out=pk2[:], in_=pB[:])
                pos8 = wk.tile([128, 8], U16, tag="pos8")
                nc.vector.max_index(out=pos8[:], in_max=pk8[:],
                                    in_values=packed[:])
                pos2 = wk.tile([128, 8], U16, tag="pos2")
                nc.vector.max_index(out=pos2[:], in_max=pk2[:],
                                    in_values=pB[:])
                pk = wk.tile([128, LP], F32, tag="pk")
                nc.vector.tensor_copy(pk[:, 0:8], pk8[:])
                nc.vector.tensor_copy(pk[:, 8:LP], pk2[:, 0:LP - 8])
                pos = wk.tile([128, LP], U16, tag="pos")
                nc.vector.tensor_copy(pos[:, 0:8], pos8[:])
                nc.vector.tensor_copy(pos[:, 8:LP], pos2[:, 0:LP - 8])
                # wv = pos >> 3 gives the window id (8 candidates per window)
                # window id from candidate position; local idx by unpacking
                wv = wk.tile([128, LP], U16, tag="wv")
                nc.vector.tensor_scalar(out=wv[:], in0=pos[:],
                                        scalar1=3, scalar2=None,
                                        op0=AL.logical_shift_right)
                woff = wk.tile([128, LP], U16, tag="woff")
                nc.vector.tensor_scalar(out=woff[:], in0=wv[:],
                                        scalar1=WIN, scalar2=None,
                                        op0=AL.mult)
                dbl = wk.tile([128, LP], F32, tag="dbl")
                nc.vector.tensor_scalar(out=dbl[:], in0=pk[:], scalar1=2.0,
                                        scalar2=None, op0=AL.mult)
                cvh = wk.tile([128, LP], F16, tag="cvh")
                nc.vector.tensor_copy(cvh[:], dbl[:])
                cvf = wk.tile([128, LP], F32, tag="cvf")
                nc.vector.tensor_copy(cvf[:], cvh[:])
                idr = wk.tile([128, LP], F32, tag="idr")
                nc.vector.tensor_tensor(out=idr[:], in0=dbl[:], in1=cvf[:],
                                        op=AL.subtract)
                lf = wk.tile([128, LP], F32, tag="lf")
                nc.vector.tensor_scalar(out=lf[:], in0=idr[:],
                                        scalar1=float(2 ** 25),
                                        scalar2=None, op0=AL.mult)
                # P = woff + local is the pooled-pair index: the pair is
                # rows (2P, 2P+1) = one [16384, 512] view row
                loc16 = wk.tile([128, LP], U16, tag="loc16")
                nc.vector.tensor_copy(loc16[:], lf[:])
                Pu16 = wk.tile([128, LP], U16, tag="Pu16")
                nc.vector.tensor_tensor(out=Pu16[:], in0=woff[:],
                                        in1=loc16[:], op=AL.add)
                Pu = wk.tile([128, LP], U32, tag="Pu")
                nc.vector.tensor_copy(Pu[:], Pu16[:])
                if DEBUG:
                    cvf_d = wk.tile([128, NCAND], F32, tag="cvf_d")
                    nc.vector.tensor_copy(cvf_d[:], cv[:])
                    nc.sync.dma_start(dbg_cv[qt], cvf_d[:])
                    nc.sync.dma_start(dbg_ci[qt], ci[:])
                    nc.sync.dma_start(dbg_rows[qt], Pu[:])

                # -- gather LP row-pairs (2KB contiguous descriptors) --
                G = gat.tile([128, LP, 2 * D], F32, tag="G")
                for j in range(LP):
                    nc.gpsimd.indirect_dma_start(
                        out=G[:, j, :], out_offset=None, in_=mem_pairs,
                        in_offset=IndirectOffsetOnAxis(ap=Pu[:, j:j + 1],
                                                       axis=0))
                return (G,)

            def refine(qt, G):
                """exact re-dot, top-8 softmax weights, PE weighted sum."""
                dots = wk.tile([128, 2 * LP], F32, tag="dots")
                scr = scrp.tile([128, D], F32, tag="scr")
                gn2 = wk.tile([128, 2 * LP], F32, tag="gn2")
                gsq = scrp.tile([128, D], F32, tag="gsq")
                for j in range(LP):
                    for h in range(2):
                        nc.vector.scalar_tensor_tensor(
                            out=scr[:], in0=G[:, j, h * D:(h + 1) * D],
                            scalar=1.0, in1=nqf[:, qt % 3, :],
                            op0=AL.mult, op1=AL.mult,
                            accum_out=dots[:, 2 * j + h:2 * j + h + 1])
                        nc.scalar.activation(
                            gsq[:], G[:, j, h * D:(h + 1) * D], AF.Square,
                            accum_out=gn2[:, 2 * j + h:2 * j + h + 1])
                gn = wk.tile([128, 2 * LP], F32, tag="gn")
                nc.scalar.activation(gn[:], gn2[:], AF.Sqrt)
                rnq = wk.tile([128, 2 * LP], F32, tag="rnq")
                nc.vector.reciprocal(rnq[:], gn[:])
                cos = wk.tile([128, 2 * LP], F32, tag="cos")
                nc.vector.tensor_tensor(out=cos[:], in0=dots[:], in1=rnq[:],
                                        op=AL.mult)
                msk = wk.tile([128, 2 * LP], F32, tag="msk")
                nc.vector.tensor_scalar(out=msk[:], in0=cos[:],
                                        scalar1=SELF_MATCH, scalar2=NEG,
                                        op0=AL.is_ge, op1=AL.mult)
                nc.vector.tensor_tensor(out=cos[:], in0=cos[:], in1=msk[:],
                                        op=AL.add)
                t8 = wk.tile([128, 8], F32, tag="t8")
                nc.vector.max(out=t8[:], in_=cos[:])
                sh = wk.tile([128, 2 * LP], F32, tag="sh")
                nc.vector.tensor_scalar(out=sh[:], in0=cos[:],
                                        scalar1=t8[:, 0:1], scalar2=None,
                                        op0=AL.subtract)
                wm = wk.tile([128, 2 * LP], F32, tag="wm")
                nc.vector.tensor_scalar(out=wm[:], in0=cos[:],
                                        scalar1=t8[:, 7:8], scalar2=None,
                                        op0=AL.is_ge)
                ex = wk.tile([128, 2 * LP], F32, tag="ex")
                nc.scalar.activation(ex[:], sh[:], AF.Exp)
                wts = wk.tile([128, 2 * LP], F32, tag="wts")
                nc.vector.tensor_tensor(out=wts[:], in0=ex[:], in1=wm[:],
                                        op=AL.mult)
                if DEBUG:
                    nc.sync.dma_start(dbg_cos[qt], cos[:])
                    nc.sync.dma_start(dbg_wts[qt], wts[:])

                acc = psA.tile([128, D], F32, tag="acc")
                for jj in range(2 * LP):
                    dg = dgp.tile([128, 128], F32, tag="dg")
                    nc.vector.tensor_scalar(
                        out=dg[:], in0=identF[:],
                        scalar1=wts[:, jj:jj + 1], scalar2=None,
                        op0=AL.mult)
                    nc.tensor.matmul(
                        acc[:], dg[:],
                        G[:, jj // 2, (jj % 2) * D:(jj % 2 + 1) * D],
                        start=(jj == 0), stop=(jj == 2 * LP - 1))
                accF = wk.tile([128, D], F32, tag="accF")
                nc.scalar.activation(accF[:], acc[:], AF.Copy)
                an2 = wk.tile([128, 1], F32, tag="an2")
                scrN = scrp.tile([128, D], F32, tag="scrN")
                nc.vector.scalar_tensor_tensor(
                    out=scrN[:], in0=accF[:], scalar=1.0, in1=accF[:],
                    op0=AL.mult, op1=AL.mult, accum_out=an2[:])
                an = wk.tile([128, 1], F32, tag="an")
                nc.scalar.activation(an[:], an2[:], AF.Sqrt)
                ar = wk.tile([128, 1], F32, tag="ar")
                nc.vector.reciprocal(ar[:], an[:])
                scl = wk.tile([128, 1], F32, tag="scl")
                nc.vector.tensor_tensor(out=scl[:], in0=ar[:],
                                        in1=qnorm[:, qt:qt + 1], op=AL.mult)
                ot = wk.tile([128, D], F32, tag="ot")
                nc.scalar.activation(ot[:], accF[:], AF.Copy, scale=scl[:])
                nc.sync.dma_start(out[qt * 128:(qt + 1) * 128, :], ot[:])

            prev = None
            for qt in range(NQT):
                cb = (lambda q=qt, p=prev: refine(q - 1, *p)) if prev else None
                prev = screen(qt, mid_cb=cb)
            refine(NQT - 1, *prev)

    return nc


_CACHED_NC = None


def _get_nc():
    global _CACHED_NC
    if _CACHED_NC is None:
        _install_patches()
        _CACHED_NC = _build()
    return _CACHED_NC


def kernel(query, memory, k):
    query = np.ascontiguousarray(np.asarray(query, dtype=np.float32))
    memory = np.ascontiguousarray(np.asarray(memory, dtype=np.float32))
    k_val = int(np.asarray(k))
    assert query.shape == (B, D) and memory.shape == (M, D), (query.shape, memory.shape)
    assert k_val == K, f"kernel compiled for k={K}, got {k_val}"

    from concourse.bass_utils import run_bass_kernel_spmd

    nc = _get_nc()
    in_maps = [
        {"q": query[i * B_LOC: (i + 1) * B_LOC], "mem": memory}
        for i in range(N_CORES)
    ]
    res = run_bass_kernel_spmd(nc, in_maps, list(range(N_CORES)))
    return np.concatenate([res.results[i]["out"] for i in range(N_CORES)], axis=0)


# revision 3
# speedup vs baseline: 1.0149x; 1.0149x over previous
"""Trainium2 Bass kernel for nn_MemoryBank_51135880626820 (scatter_memory), v3.

Data-parallel over the query batch across 8 NeuronCores: the [32768, 256]
memory bank is replicated per core, each core handles 1024 query rows.

Per-core pipeline (v3):
  - prep: per 1024-row bank chunk: DMA, scalar row-norms (Square+accum) +
    Rsqrt + normalize-cast to bf16, PE transpose to d-major resident bank
    [128, 2, 32768] bf16, DVE 2x-mode PSUM evacuation. norm^2 per row saved
    to DRAM scratch for the exact-refine gather.
  - screen per qtile (128 queries): 32 supers of 1024 m-cols: 2 accumulating
    bf16 matmuls (stationary = transposed queries, reused) -> PSUM f32;
    DVE 2:1 max-pool of adjacent column pairs -> bf16 pooled [128, 16384];
    DVE max8 + find8 per 2048-pooled window -> 64 candidates (value + local
    idx); candidates packed exactly into f32 as  v*0.5 + gidx*2^-26  so the
    top-10 prune (max8 + match_replace + max8) carries indices for free.
  - refine per qtile: one batched indirect DMA gathers both rows of each
    pooled pair (2KB contiguous descriptors via a [16384, 512] view of the
    bank) + their norm^2; exact f32 re-dot via fused tensor_tensor_reduce;
    self-match mask, top-8, unnormalized softmax weights.
  - weighted sum on PE: per candidate a diagonal-stationary f32r matmul
    accumulating w_j * row_j into PSUM; scalar rescales to ||q|| (softmax
    denominator cancels against the renormalize).

self-contained: hardcodes all shapes; builds and caches the Bass program on
first call.
"""

import sys

for _p in ("/opt/trn_rl_repo",):
    if _p not in sys.path:
        sys.path.insert(0, _p)

import numpy as np

import concourse.bass as bass
import concourse.mybir as mybir
import concourse.tile as tile
from concourse.bass import IndirectOffsetOnAxis
from concourse.masks import make_identity

F32 = mybir.dt.float32
F32R = mybir.dt.float32r
BF16 = mybir.dt.bfloat16
F16 = mybir.dt.float16
U16 = mybir.dt.uint16
U32 = mybir.dt.uint32

N_CORES = 8
B = 8192
B_LOC = B // N_CORES        # 1024
M = 32768
D = 256
K = 8
NQT = B_LOC // 128          # 8 query tiles per core
SUP = 1024                  # matmul super (PSUM tile) width in m columns
NSUP = M // SUP             # 32
PW = SUP // 2               # 512 pooled elems per super
NPOOL = M // 2              # 16384 pooled elems per qtile
WIN = 1024                  # pooled window for max8/find8
NW = NPOOL // WIN           # 8 windows
NCAND = NW * 8              # 64 candidates per qtile
LP = 11                     # pruned candidates (pairs) -> 22 gathered rows
NEG = -1.0e30
SELF_MATCH = 0.9999
EPACK = 2.0 ** -26          # pooled-index packing epsilon (on v*0.5)
DEBUG = False               # adds per-qtile intermediate dumps as outputs


# --------------------------------------------------------------------------
# workarounds for this container's walrus build, which rejects more than one
# sync-wait per instruction ("Too many sync wait commands").
# --------------------------------------------------------------------------
def _install_patches():
    import json

    import bass_rust
    import concourse.bass_utils as _bu
    import concourse.bass2jax as _b2j
    import concourse.tile as tile_mod
    from concourse.tile import TileContext

    if getattr(_bu, "_mb_patched", False):
        return

    try:
        ScopedClock = tile_mod.ScopedClock
    except AttributeError:
        ScopedClock = bass_rust.ScopedClock

    def _patched_drain_and_barrier(self, tick_clock, wait_clock):
        nc = self.nc
        drain_inst = nc.sync.drain()
        wait_clock.add_sem_waits(
            drain_inst.ins, ScopedClock({None: tick_clock.global_clock})
        )
        si = drain_inst.ins.sync_info
        waits = list(si.on_wait) if si is not None and si.on_wait else []
        if len(waits) > 1:
            drain_inst.ins.sync_info = bass_rust.SyncInfo(
                on_wait=[waits[0]],
                on_update=list(si.on_update) if si.on_update else [],
            )
            for w in waits[1:]:
                nop = nc.sync.nop(nofuse=True, hint="tail_wait")
                nop.ins.sync_info = bass_rust.SyncInfo(on_wait=[w], on_update=[])
        nc.all_engine_barrier()
        assert self.sems is not None
        popped = nc._tile_sem_poison_stack.pop()
        assert popped is self._sem_poison
        nc.clear_and_free_semaphores(list(self.sems.allocated().values()))
        nc.all_engine_barrier()

    TileContext._drain_and_barrier = _patched_drain_and_barrier

    def dedup_ldweights(m):
        # consecutive Ldweights with identical operands on the PE stream
        # (only Matmult/NoOp between) reload the same stationary; turn the
        # duplicates into NoOps (keeping their sync waits/updates).
        for fn in m.get("functions", []):
            for bb in fn.get("blocks", []):
                last_key = None
                for ins in bb.get("instructions", []):
                    op = ins.get("opcode")
                    if op == "Ldweights":
                        key = json.dumps(
                            [ins.get("ins"), ins.get("perf_mode"),
                             ins.get("is_transpose"), ins.get("tile_position")],
                            sort_keys=True)
                        if key == last_key:
                            ins["opcode"] = "NoOp"
                            ins["ins"] = []
                            ins["outs"] = []
                            ins["text_hint"] = "dedup_ldw"
                        else:
                            last_key = key
                    elif op in ("Matmult", "NoOp"):
                        pass
                    elif ins.get("engine") == "PE":
                        last_key = None
        return m

    import os as _os

    def split_multiwaits(bir_json):
        m = json.loads(bir_json)
        changed = True
        for fn in m.get("functions", []):
            for bb in fn.get("blocks", []):
                insts = bb.get("instructions", [])
                out = []
                for ins in insts:
                    si = ins.get("sync_info") or {}
                    waits = si.get("on_wait") or []
                    if len(waits) > 1:
                        changed = True
                        for kk, w in enumerate(waits[:-1]):
                            out.append({
                                "debug": ins.get("debug", 0),
                                "engine": ins["engine"],
                                "ins": [],
                                "name": f"{ins['name']}-w{kk}",
                                "opcode": "NoOp",
                                "outs": [],
                                "sync_info": {"on_update": [], "on_wait": [w]},
                                "text_hint": "split_wait",
                            })
                        si = dict(si)
                        si["on_wait"] = [waits[-1]]
                        ins = dict(ins)
                        ins["sync_info"] = si
                    out.append(ins)
                bb["instructions"] = out
        return json.dumps(m).encode()

    _orig_compile = _bu.compile_bir_kernel

    def _patched_compile(bir_json, tmpdir, neff_name="file.neff"):
        if isinstance(bir_json, str):
            bir_json = bir_json.encode()
        return _orig_compile(split_multiwaits(bir_json), tmpdir, neff_name)

    _bu.compile_bir_kernel = _patched_compile
    _b2j.compile_bir_kernel = _patched_compile

    _bu._mb_patched = True


# --------------------------------------------------------------------------
# per-core Bass program
# --------------------------------------------------------------------------
def _build():
    nc = bass.Bass("TRN2", target_bir_lowering=False, debug=False)
    q_in = nc.dram_tensor("q", [B_LOC, D], F32, kind="ExternalInput")
    mem_in = nc.dram_tensor("mem", [M, D], F32, kind="ExternalInput")
    out = nc.dram_tensor("out", [B_LOC, D], F32, kind="ExternalOutput")
    if DEBUG:
        dbg_cv = nc.dram_tensor("dbg_cv", [NQT, 128, NCAND], F32, kind="ExternalOutput")
        dbg_ci = nc.dram_tensor("dbg_ci", [NQT, 128, NCAND], U16, kind="ExternalOutput")
        dbg_rows = nc.dram_tensor("dbg_rows", [NQT, 128, 2 * LP], U32, kind="ExternalOutput")
        dbg_cos = nc.dram_tensor("dbg_cos", [NQT, 128, 2 * LP], F32, kind="ExternalOutput")
        dbg_wts = nc.dram_tensor("dbg_wts", [NQT, 128, 2 * LP], F32, kind="ExternalOutput")

    AF = mybir.ActivationFunctionType
    AL = mybir.AluOpType

    with tile.TileContext(nc) as tc, tc.tile_pool(name="res", bufs=1) as res:
        bankT = res.tile([128, 2, M], BF16, tag="bankT")    # d-major bank
        identB = res.tile([128, 128], BF16, tag="identB")
        identF = res.tile([128, 128], F32, tag="identF")
        make_identity(nc, identB[:])
        make_identity(nc, identF[:])
        nqf = res.tile([128, 3, D], F32, tag="nqf")         # exact normalized q (rotating slots)
        qT = res.tile([128, NQT, 2, 128], BF16, tag="qT")   # stationary queries
        qnorm = res.tile([128, NQT], F32, tag="qnorm")

        # ---- prep: bank chunks of 1024 rows ----
        with (
            tc.tile_pool(name="prep", bufs=4) as pp,
            tc.tile_pool(name="ppsum", bufs=3, space="PSUM") as ppsum,
        ):
            for c in range(M // 1024):
                mt = pp.tile([128, 8, D], F32, tag="mt")
                nc.sync.dma_start(
                    mt[:],
                    mem_in[c * 1024:(c + 1) * 1024, :]
                        .rearrange("(g p) d -> p g d", p=128))
                n2 = pp.tile([128, 8], F32, tag="n2")
                sqs = pp.tile([128, D], F32, tag="sqs")
                sqv = pp.tile([128, D], F32, tag="sqv")
                for g in range(4):
                    nc.scalar.activation(sqs[:], mt[:, g, :], AF.Square,
                                         accum_out=n2[:, g:g + 1])
                for g in range(4, 8):
                    nc.vector.scalar_tensor_tensor(
                        out=sqv[:], in0=mt[:, g, :], scalar=1.0,
                        in1=mt[:, g, :], op0=AL.mult, op1=AL.mult,
                        accum_out=n2[:, g:g + 1])
                nrm = pp.tile([128, 8], F32, tag="nrm")
                nc.scalar.activation(nrm[:], n2[:], AF.Sqrt)
                rn = pp.tile([128, 8], F32, tag="rn")
                nc.vector.reciprocal(rn[:], nrm[:])
                nmb = pp.tile([128, 8, D], BF16, tag="nmb")
                nc.gpsimd.tensor_tensor(
                    out=nmb[:], in0=mt[:],
                    in1=rn[:].rearrange("p (g o) -> p g o", o=1)
                        .to_broadcast([128, 8, D]),
                    op=AL.mult)
                for h in range(2):
                    pt = ppsum.tile([128, 1024], BF16, tag="pt")
                    for g in range(8):
                        nc.tensor.transpose(
                            pt[:, g * 128:(g + 1) * 128],
                            nmb[:, g, h * 128:(h + 1) * 128], identB[:])
                    nc.vector.tensor_copy(
                        bankT[:, h, c * 1024:(c + 1) * 1024], pt[:])

        # ---- main: screen(qt)/refine(qt-1) software-pipelined ----
        with (
            tc.tile_pool(name="wk", bufs=2) as wk,
            tc.tile_pool(name="scrp", bufs=1) as scrp,
            tc.tile_pool(name="pool2", bufs=2) as pool2,
            tc.tile_pool(name="gat", bufs=2) as gat,
            tc.tile_pool(name="gbp", bufs=1) as gbp,
            tc.tile_pool(name="dgp", bufs=2) as dgp,
            tc.tile_pool(name="psS", bufs=3, space="PSUM") as psS,
            tc.tile_pool(name="psQ", bufs=1, space="PSUM") as psQ,
            tc.tile_pool(name="psA", bufs=1, space="PSUM") as psA,
        ):
            mem_pairs = mem_in[:].rearrange("(P two) d -> P (two d)", two=2)

            def screen(qt, mid_cb=None):
                """matmul + pool + window top8 + prune + gather issue.
                Returns state needed by refine(qt)."""
                # -- query prep --
                qf = wk.tile([128, D], F32, tag="qf")
                nc.sync.dma_start(qf[:], q_in[qt * 128:(qt + 1) * 128, :])
                qn2 = wk.tile([128, 1], F32, tag="qn2")
                qsq = scrp.tile([128, D], F32, tag="qsq")
                nc.scalar.activation(qsq[:], qf[:], AF.Square,
                                     accum_out=qn2[:])
                nc.scalar.activation(qnorm[:, qt:qt + 1], qn2[:], AF.Sqrt)
                qr = wk.tile([128, 1], F32, tag="qr")
                nc.vector.reciprocal(qr[:], qnorm[:, qt:qt + 1])
                nc.scalar.activation(nqf[:, qt % 3, :], qf[:], AF.Copy,
                                     scale=qr[:])
                nqb = wk.tile([128, D], BF16, tag="nqb")
                nc.vector.tensor_copy(nqb[:], nqf[:, qt % 3, :])
                pq = psQ.tile([128, 256], BF16, tag="pq")
                for h in range(2):
                    nc.tensor.transpose(pq[:, h * 128:(h + 1) * 128],
                                        nqb[:, h * 128:(h + 1) * 128],
                                        identB[:])
                nc.vector.tensor_copy(
                    qT[:, qt, :, :],
                    pq[:].rearrange("p (h j) -> p h j", h=2))

                # -- matmul + pool + window screen --
                cv = wk.tile([128, NCAND], F16, tag="cv")
                ci = wk.tile([128, NCAND], U16, tag="ci")
                for w in range(NW):
                    if w == 4 and mid_cb is not None:
                        mid_cb()
                    pl = pool2.tile([128, WIN], F16, tag="pl")
                    ps0 = psS.tile([128, SUP], F32, tag="ps")
                    ps1 = psS.tile([128, SUP], F32, tag="ps")
                    pst = [ps0, ps1]
                    # h-outer over a super pair: one stationary load
                    # covers 4 consecutive matmuls (deduped in BIR).
                    for h in range(2):
                        for si in range(2):
                            s = w * 2 + si
                            for b2 in range(2):
                                nc.tensor.matmul(
                                    pst[si][:, b2 * 512:(b2 + 1) * 512],
                                    qT[:, qt, h, :],
                                    bankT[:, h, s * SUP + b2 * 512:
                                          s * SUP + (b2 + 1) * 512],
                                    start=(h == 0), stop=(h == 1))
                    # DVE TT may read at most one PSUM operand: scalar
                    # evacuates the odd elements, DVE maxes vs the evens.
                    for si in range(2):
                        psv = pst[si][:].rearrange(
                            "p (m two) -> p two m", two=2)
                        hb = wk.tile([128, PW], F16, tag="hb")
                        nc.scalar.activation(hb[:], psv[:, 1, :], AF.Copy)
                        nc.vector.tensor_tensor(
                            out=pl[:, si * PW:(si + 1) * PW],
                            in0=psv[:, 0, :], in1=hb[:], op=AL.max)
                    nc.vector.max(out=cv[:, w * 8:(w + 1) * 8], in_=pl[:])
                    nc.vector.max_index(
                        out=ci[:, w * 8:(w + 1) * 8],
                        in_max=cv[:, w * 8:(w + 1) * 8], in_values=pl[:])

                # -- prune to LP: pack (value*0.5 + local_idx*2^-26) --
                cif = wk.tile([128, NCAND], F32, tag="cif")
                nc.vector.tensor_copy(cif[:], ci[:])
                packed = wk.tile([128, NCAND], F32, tag="packed")
                nc.vector.tensor_scalar(out=packed[:], in0=cif[:],
                                        scalar1=EPACK, scalar2=None,
                                        op0=AL.mult)
                hcv = wk.tile([128, NCAND], F32, tag="hcv")
                nc.vector.tensor_scalar(out=hcv[:], in0=cv[:], scalar1=0.5,
                                        scalar2=None, op0=AL.mult)
                nc.vector.tensor_tensor(out=packed[:], in0=packed[:],
                                        in1=hcv[:], op=AL.add)
                pk8 = wk.tile([128, 8], F32, tag="pk8")
                nc.vector.max(out=pk8[:], in_=packed[:])
                pB = wk.tile([128, NCAND], F32, tag="pB")
                nc.vector.match_replace(out=pB[:], in_to_replace=pk8[:],
                                        in_values=packed[:], imm_value=NEG)
                pk2 = wk.tile([128, 8], F32, tag="pk2")
                nc.vector.max(out=pk2[:], in_=pB[:])
                pos8 = wk.tile([128, 8], U16, tag="pos8")
                nc.vector.max_index(out=pos8[:], in_max=pk8[:],
                                    in_values=packed[:])
                pos2 = wk.tile([128, 8], U16, tag="pos2")
                nc.vector.max_index(out=pos2[:], in_max=pk2[:],
                                    in_values=pB[:])
                pk = wk.tile([128, LP], F32, tag="pk")
                nc.vector.tensor_copy(pk[:, 0:8], pk8[:])
                nc.vector.tensor_copy(pk[:, 8:LP], pk2[:, 0:LP - 8])
                pos = wk.tile([128, LP], U16, tag="pos")
                nc.vector.tensor_copy(pos[:, 0:8], pos8[:])
                nc.vector.tensor_copy(pos[:, 8:LP], pos2[:, 0:LP - 8])
                # wv = pos >> 3 gives the window id (8 candidates per window)
                # window id from candidate position; local idx by unpacking
                wv = wk.tile([128, LP], U16, tag="wv")
                nc.vector.tensor_scalar(out=wv[:], in0=pos[:],
                                        scalar1=3, scalar2=None,
                                        op0=AL.logical_shift_right)
                woff = wk.tile([128, LP], U16, tag="woff")
                nc.vector.tensor_scalar(out=woff[:], in0=wv[:],
                                        scalar1=WIN, scalar2=None,
                                        op0=AL.mult)
                dbl = wk.tile([128, LP], F32, tag="dbl")
                nc.vector.tensor_scalar(out=dbl[:], in0=pk[:], scalar1=2.0,
                                        scalar2=None, op0=AL.mult)
                cvh = wk.tile([128, LP], F16, tag="cvh")
                nc.vector.tensor_copy(cvh[:], dbl[:])
                cvf = wk.tile([128, LP], F32, tag="cvf")
                nc.vector.tensor_copy(cvf[:], cvh[:])
                idr = wk.tile([128, LP], F32, tag="idr")
                nc.vector.tensor_tensor(out=idr[:], in0=dbl[:], in1=cvf[:],
                                        op=AL.subtract)
                lf = wk.tile([128, LP], F32, tag="lf")
                nc.vector.tensor_scalar(out=lf[:], in0=idr[:],
                                        scalar1=float(2 ** 25),
                                        scalar2=None, op0=AL.mult)
                # P = woff + local is the pooled-pair index: the pair is
                # rows (2P, 2P+1) = one [16384, 512] view row
                loc16 = wk.tile([128, LP], U16, tag="loc16")
                nc.vector.tensor_copy(loc16[:], lf[:])
                Pu16 = wk.tile([128, LP], U16, tag="Pu16")
                nc.vector.tensor_tensor(out=Pu16[:], in0=woff[:],
                                        in1=loc16[:], op=AL.add)
                Pu = wk.tile([128, LP], U32, tag="Pu")
                nc.vector.tensor_copy(Pu[:], Pu16[:])
                if DEBUG:
                    cvf_d = wk.tile([128, NCAND], F32, tag="cvf_d")
                    nc.vector.tensor_copy(cvf_d[:], cv[:])
                    nc.sync.dma_start(dbg_cv[qt], cvf_d[:])
                    nc.sync.dma_start(dbg_ci[qt], ci[:])
                    nc.sync.dma_start(dbg_rows[qt], Pu[:])

                # -- gather LP row-pairs (2KB contiguous descriptors) --
                G = gat.tile([128, LP, 2 * D], F32, tag="G")
                for j in range(LP):
                    nc.gpsimd.indirect_dma_start(
                        out=G[:, j, :], out_offset=None, in_=mem_pairs,
                        in_offset=IndirectOffsetOnAxis(ap=Pu[:, j:j + 1],
                                                       axis=0))
                return (G,)

            def refine(qt, G):
                """exact re-dot, top-8 softmax weights, PE weighted sum."""
                dots = wk.tile([128, 2 * LP], F32, tag="dots")
                scr = scrp.tile([128, D], F32, tag="scr")
                gn2 = wk.tile([128, 2 * LP], F32, tag="gn2")
                gsq = scrp.tile([128, D], F32, tag="gsq")
                for j in range(LP):
                    for h in range(2):
                        nc.vector.scalar_tensor_tensor(
                            out=scr[:],
                            in0=G[:, j, h * D:(h + 1) * D],
                            scalar=1.0, in1=nqf[:, qt % 3, :],
                            op0=AL.mult, op1=AL.mult,
                            accum_out=dots[:, 2 * j + h:2 * j + h + 1])
                        nc.scalar.activation(
                            gsq[:], G[:, j, h * D:(h + 1) * D], AF.Square,
                            accum_out=gn2[:, 2 * j + h:2 * j + h + 1])
                gn = wk.tile([128, 2 * LP], F32, tag="gn")
                nc.scalar.activation(gn[:], gn2[:], AF.Sqrt)
                rnq = wk.tile([128, 2 * LP], F32, tag="rnq")
                nc.vector.reciprocal(rnq[:], gn[:])
                cos = wk.tile([128, 2 * LP], F32, tag="cos")
                nc.vector.tensor_tensor(out=cos[:], in0=dots[:], in1=rnq[:],
                                        op=AL.mult)
                msk = wk.tile([128, 2 * LP], F32, tag="msk")
                nc.vector.tensor_scalar(out=msk[:], in0=cos[:],
                                        scalar1=SELF_MATCH, scalar2=NEG,
                                        op0=AL.is_ge, op1=AL.mult)
                nc.vector.tensor_tensor(out=cos[:], in0=cos[:], in1=msk[:],
                                        op=AL.add)
                t8 = wk.tile([128, 8], F32, tag="t8")
                nc.vector.max(out=t8[:], in_=cos[:])
                sh = wk.tile([128, 2 * LP], F32, tag="sh")
                nc.vector.tensor_scalar(out=sh[:], in0=cos[:],
                                        scalar1=t8[:, 0:1], scalar2=None,
                                        op0=AL.subtract)
                wm = wk.tile([128, 2 * LP], F32, tag="wm")
                nc.vector.tensor_scalar(out=wm[:], in0=cos[:],
                                        scalar1=t8[:, 7:8], scalar2=None,
                                        op0=AL.is_ge)
                ex = wk.tile([128, 2 * LP], F32, tag="ex")
                nc.scalar.activation(ex[:], sh[:], AF.Exp)
                wts = wk.tile([128, 2 * LP], F32, tag="wts")
                nc.vector.tensor_tensor(out=wts[:], in0=ex[:], in1=wm[:],
                                        op=AL.mult)
                if DEBUG:
                    nc.sync.dma_start(dbg_cos[qt], cos[:])
                    nc.sync.dma_start(dbg_wts[qt], wts[:])

                acc = psA.tile([128, D], F32, tag="acc")
                for jj in range(2 * LP):
                    dg = dgp.tile([128, 128], F32, tag="dg")
                    nc.scalar.activation(dg[:], identF[:], AF.Copy,
                                         scale=wts[:, jj:jj + 1])
                    nc.tensor.matmul(
                        acc[:], dg[:],
                        G[:, jj // 2, (jj % 2) * D:(jj % 2 + 1) * D],
                        start=(jj == 0), stop=(jj == 2 * LP - 1))
                accF = wk.tile([128, D], F32, tag="accF")
                nc.scalar.activation(accF[:], acc[:], AF.Copy)
                an2 = wk.tile([128, 1], F32, tag="an2")
                scrN = scrp.tile([128, D], F32, tag="scrN")
                nc.vector.scalar_tensor_tensor(
                    out=scrN[:], in0=accF[:], scalar=1.0, in1=accF[:],
                    op0=AL.mult, op1=AL.mult, accum_out=an2[:])
                an = wk.tile([128, 1], F32, tag="an")
                nc.scalar.activation(an[:], an2[:], AF.Sqrt)
                ar = wk.tile([128, 1], F32, tag="ar")
                nc.vector.reciprocal(ar[:], an[:])
                scl = wk.tile([128, 1], F32, tag="scl")
                nc.vector.tensor_tensor(out=scl[:], in0=ar[:],
                                        in1=qnorm[:, qt:qt + 1], op=AL.mult)
                ot = wk.tile([128, D], F32, tag="ot")
                nc.scalar.activation(ot[:], accF[:], AF.Copy, scale=scl[:])
                nc.sync.dma_start(out[qt * 128:(qt + 1) * 128, :], ot[:])

            prev = None
            for qt in range(NQT):
                cb = (lambda q=qt, p=prev: refine(q - 1, *p)) if prev else None
                prev = screen(qt, mid_cb=cb)
            refine(NQT - 1, *prev)

    return nc


_CACHED_NC = None


def _get_nc():
    global _CACHED_NC
    if _CACHED_NC is None:
        _install_patches()
        _CACHED_NC = _build()
    return _CACHED_NC


def kernel(query, memory, k):
    query = np.ascontiguousarray(np.asarray(query, dtype=np.float32))
    memory = np.ascontiguousarray(np.asarray(memory, dtype=np.float32))
    k_val = int(np.asarray(k))
    assert query.shape == (B, D) and memory.shape == (M, D), (query.shape, memory.shape)
    assert k_val == K, f"kernel compiled for k={K}, got {k_val}"

    from concourse.bass_utils import run_bass_kernel_spmd

    nc = _get_nc()
    in_maps = [
        {"q": query[i * B_LOC: (i + 1) * B_LOC], "mem": memory}
        for i in range(N_CORES)
    ]
    res = run_bass_kernel_spmd(nc, in_maps, list(range(N_CORES)))
    return np.concatenate([res.results[i]["out"] for i in range(N_CORES)], axis=0)


# revision 4
# speedup vs baseline: 1.0420x; 1.0267x over previous
"""Trainium2 Bass kernel for nn_MemoryBank_51135880626820 (scatter_memory), v3.

Data-parallel over the query batch across 8 NeuronCores: the [32768, 256]
memory bank is replicated per core, each core handles 1024 query rows.

Per-core pipeline (v3):
  - prep: per 1024-row bank chunk: DMA, scalar row-norms (Square+accum) +
    Rsqrt + normalize-cast to bf16, PE transpose to d-major resident bank
    [128, 2, 32768] bf16, DVE 2x-mode PSUM evacuation. norm^2 per row saved
    to DRAM scratch for the exact-refine gather.
  - screen per qtile (128 queries): 32 supers of 1024 m-cols: 2 accumulating
    bf16 matmuls (stationary = transposed queries, reused) -> PSUM f32;
    DVE 2:1 max-pool of adjacent column pairs -> bf16 pooled [128, 16384];
    DVE max8 + find8 per 2048-pooled window -> 64 candidates (value + local
    idx); candidates packed exactly into f32 as  v*0.5 + gidx*2^-26  so the
    top-10 prune (max8 + match_replace + max8) carries indices for free.
  - refine per qtile: one batched indirect DMA gathers both rows of each
    pooled pair (2KB contiguous descriptors via a [16384, 512] view of the
    bank) + their norm^2; exact f32 re-dot via fused tensor_tensor_reduce;
    self-match mask, top-8, unnormalized softmax weights.
  - weighted sum on PE: per candidate a diagonal-stationary f32r matmul
    accumulating w_j * row_j into PSUM; scalar rescales to ||q|| (softmax
    denominator cancels against the renormalize).

self-contained: hardcodes all shapes; builds and caches the Bass program on
first call.
"""

import sys

for _p in ("/opt/trn_rl_repo",):
    if _p not in sys.path:
        sys.path.insert(0, _p)

import numpy as np

import concourse.bass as bass
import concourse.mybir as mybir
import concourse.tile as tile
from concourse.bass import IndirectOffsetOnAxis
from concourse.masks import make_identity

F32 = mybir.dt.float32
F32R = mybir.dt.float32r
BF16 = mybir.dt.bfloat16
F16 = mybir.dt.float16
U16 = mybir.dt.uint16
U32 = mybir.dt.uint32

N_CORES = 8
B = 8192
B_LOC = B // N_CORES        # 1024
M = 32768
D = 256
K = 8
NQT = B_LOC // 128          # 8 query tiles per core
SUP = 1024                  # matmul super (PSUM tile) width in m columns
NSUP = M // SUP             # 32
PW = SUP // 2               # 512 pooled elems per super
NPOOL = M // 2              # 16384 pooled elems per qtile
WIN = 1024                  # pooled window for max8/find8
NW = NPOOL // WIN           # 8 windows
NCAND = NW * 8              # 64 candidates per qtile
LP = 11                     # pruned candidates (pairs) -> 22 gathered rows
NEG = -1.0e30
SELF_MATCH = 0.9999
EPACK = 2.0 ** -26          # pooled-index packing epsilon (on v*0.5)
DEBUG = False               # adds per-qtile intermediate dumps as outputs


# --------------------------------------------------------------------------
# workarounds for this container's walrus build, which rejects more than one
# sync-wait per instruction ("Too many sync wait commands").
# --------------------------------------------------------------------------
def _install_patches():
    import json

    import bass_rust
    import concourse.bass_utils as _bu
    import concourse.bass2jax as _b2j
    import concourse.tile as tile_mod
    from concourse.tile import TileContext

    if getattr(_bu, "_mb_patched", False):
        return

    try:
        ScopedClock = tile_mod.ScopedClock
    except AttributeError:
        ScopedClock = bass_rust.ScopedClock

    def _patched_drain_and_barrier(self, tick_clock, wait_clock):
        nc = self.nc
        drain_inst = nc.sync.drain()
        wait_clock.add_sem_waits(
            drain_inst.ins, ScopedClock({None: tick_clock.global_clock})
        )
        si = drain_inst.ins.sync_info
        waits = list(si.on_wait) if si is not None and si.on_wait else []
        if len(waits) > 1:
            drain_inst.ins.sync_info = bass_rust.SyncInfo(
                on_wait=[waits[0]],
                on_update=list(si.on_update) if si.on_update else [],
            )
            for w in waits[1:]:
                nop = nc.sync.nop(nofuse=True, hint="tail_wait")
                nop.ins.sync_info = bass_rust.SyncInfo(on_wait=[w], on_update=[])
        nc.all_engine_barrier()
        assert self.sems is not None
        popped = nc._tile_sem_poison_stack.pop()
        assert popped is self._sem_poison
        nc.clear_and_free_semaphores(list(self.sems.allocated().values()))
        nc.all_engine_barrier()

    TileContext._drain_and_barrier = _patched_drain_and_barrier

    def dedup_ldweights(m):
        # consecutive Ldweights with identical operands on the PE stream
        # (only Matmult/NoOp between) reload the same stationary; turn the
        # duplicates into NoOps (keeping their sync waits/updates).
        for fn in m.get("functions", []):
            for bb in fn.get("blocks", []):
                last_key = None
                for ins in bb.get("instructions", []):
                    op = ins.get("opcode")
                    if op == "Ldweights":
                        key = json.dumps(
                            [ins.get("ins"), ins.get("perf_mode"),
                             ins.get("is_transpose"), ins.get("tile_position")],
                            sort_keys=True)
                        if key == last_key:
                            ins["opcode"] = "NoOp"
                            ins["ins"] = []
                            ins["outs"] = []
                            ins["text_hint"] = "dedup_ldw"
                        else:
                            last_key = key
                    elif op in ("Matmult", "NoOp"):
                        pass
                    elif ins.get("engine") == "PE":
                        last_key = None
        return m

    import os as _os

    def split_multiwaits(bir_json):
        m = json.loads(bir_json)
        changed = True
        for fn in m.get("functions", []):
            for bb in fn.get("blocks", []):
                insts = bb.get("instructions", [])
                out = []
                for ins in insts:
                    si = ins.get("sync_info") or {}
                    waits = si.get("on_wait") or []
                    if len(waits) > 1:
                        changed = True
                        for kk, w in enumerate(waits[:-1]):
                            out.append({
                                "debug": ins.get("debug", 0),
                                "engine": ins["engine"],
                                "ins": [],
                                "name": f"{ins['name']}-w{kk}",
                                "opcode": "NoOp",
                                "outs": [],
                                "sync_info": {"on_update": [], "on_wait": [w]},
                                "text_hint": "split_wait",
                            })
                        si = dict(si)
                        si["on_wait"] = [waits[-1]]
                        ins = dict(ins)
                        ins["sync_info"] = si
                    out.append(ins)
                bb["instructions"] = out
        return json.dumps(m).encode()

    _orig_compile = _bu.compile_bir_kernel

    def _patched_compile(bir_json, tmpdir, neff_name="file.neff"):
        if isinstance(bir_json, str):
            bir_json = bir_json.encode()
        return _orig_compile(split_multiwaits(bir_json), tmpdir, neff_name)

    _bu.compile_bir_kernel = _patched_compile
    _b2j.compile_bir_kernel = _patched_compile

    _bu._mb_patched = True


# --------------------------------------------------------------------------
# per-core Bass program
# --------------------------------------------------------------------------
def _build():
    nc = bass.Bass("TRN2", target_bir_lowering=False, debug=False)
    q_in = nc.dram_tensor("q", [B_LOC, D], F32, kind="ExternalInput")
    mem_in = nc.dram_tensor("mem", [M, D], F32, kind="ExternalInput")
    out = nc.dram_tensor("out", [B_LOC, D], F32, kind="ExternalOutput")
    if DEBUG:
        dbg_cv = nc.dram_tensor("dbg_cv", [NQT, 128, NCAND], F32, kind="ExternalOutput")
        dbg_ci = nc.dram_tensor("dbg_ci", [NQT, 128, NCAND], U16, kind="ExternalOutput")
        dbg_rows = nc.dram_tensor("dbg_rows", [NQT, 128, 2 * LP], U32, kind="ExternalOutput")
        dbg_cos = nc.dram_tensor("dbg_cos", [NQT, 128, 2 * LP], F32, kind="ExternalOutput")
        dbg_wts = nc.dram_tensor("dbg_wts", [NQT, 128, 2 * LP], F32, kind="ExternalOutput")

    AF = mybir.ActivationFunctionType
    AL = mybir.AluOpType

    with tile.TileContext(nc) as tc, tc.tile_pool(name="res", bufs=1) as res:
        bankT = res.tile([128, 2, M], BF16, tag="bankT")    # d-major bank
        identB = res.tile([128, 128], BF16, tag="identB")
        identF = res.tile([128, 128], F32, tag="identF")
        make_identity(nc, identB[:])
        make_identity(nc, identF[:])
        nqf = res.tile([128, 3, D], F32, tag="nqf")         # exact normalized q (rotating slots)
        qT = res.tile([128, NQT, 2, 128], BF16, tag="qT")   # stationary queries
        qnorm = res.tile([128, NQT], F32, tag="qnorm")

        # ---- prep: bank chunks of 1024 rows ----
        with (
            tc.tile_pool(name="prep", bufs=4) as pp,
            tc.tile_pool(name="ppsum", bufs=4, space="PSUM") as ppsum,
        ):
            for c in range(M // 1024):
                mt = pp.tile([128, 8, D], F32, tag="mt")
                nc.sync.dma_start(
                    mt[:],
                    mem_in[c * 1024:(c + 1) * 1024, :]
                        .rearrange("(g p) d -> p g d", p=128))
                n2 = pp.tile([128, 8], F32, tag="n2")
                sqs = pp.tile([128, D], F32, tag="sqs")
                sqv = pp.tile([128, D], F32, tag="sqv")
                for g in range(4):
                    nc.scalar.activation(sqs[:], mt[:, g, :], AF.Square,
                                         accum_out=n2[:, g:g + 1])
                for g in range(4, 8):
                    nc.vector.scalar_tensor_tensor(
                        out=sqv[:], in0=mt[:, g, :], scalar=1.0,
                        in1=mt[:, g, :], op0=AL.mult, op1=AL.mult,
                        accum_out=n2[:, g:g + 1])
                nrm = pp.tile([128, 8], F32, tag="nrm")
                nc.scalar.activation(nrm[:], n2[:], AF.Sqrt)
                rn = pp.tile([128, 8], F32, tag="rn")
                nc.vector.reciprocal(rn[:], nrm[:])
                nmb = pp.tile([128, 8, D], BF16, tag="nmb")
                nc.gpsimd.tensor_tensor(
                    out=nmb[:], in0=mt[:],
                    in1=rn[:].rearrange("p (g o) -> p g o", o=1)
                        .to_broadcast([128, 8, D]),
                    op=AL.mult)
                for h in range(2):
                    pt = ppsum.tile([128, 1024], BF16, tag="pt")
                    for g in range(8):
                        nc.tensor.transpose(
                            pt[:, g * 128:(g + 1) * 128],
                            nmb[:, g, h * 128:(h + 1) * 128], identB[:])
                    nc.vector.tensor_copy(
                        bankT[:, h, c * 1024:(c + 1) * 1024], pt[:])

        # ---- main: screen(qt)/refine(qt-1) software-pipelined ----
        with (
            tc.tile_pool(name="wk", bufs=2) as wk,
            tc.tile_pool(name="scrp", bufs=1) as scrp,
            tc.tile_pool(name="pool2", bufs=2) as pool2,
            tc.tile_pool(name="gat", bufs=2) as gat,
            tc.tile_pool(name="gbp", bufs=1) as gbp,
            tc.tile_pool(name="dgp", bufs=2) as dgp,
            tc.tile_pool(name="psS", bufs=3, space="PSUM") as psS,
            tc.tile_pool(name="psQ", bufs=1, space="PSUM") as psQ,
            tc.tile_pool(name="psA", bufs=1, space="PSUM") as psA,
        ):
            mem_pairs = mem_in[:].rearrange("(P two) d -> P (two d)", two=2)

            def screen(qt, mid_cb=None):
                """matmul + pool + window top8 + prune + gather issue.
                Returns state needed by refine(qt)."""
                # -- query prep --
                qf = wk.tile([128, D], F32, tag="qf")
                nc.sync.dma_start(qf[:], q_in[qt * 128:(qt + 1) * 128, :])
                qn2 = wk.tile([128, 1], F32, tag="qn2")
                qsq = scrp.tile([128, D], F32, tag="qsq")
                nc.scalar.activation(qsq[:], qf[:], AF.Square,
                                     accum_out=qn2[:])
                nc.scalar.activation(qnorm[:, qt:qt + 1], qn2[:], AF.Sqrt)
                qr = wk.tile([128, 1], F32, tag="qr")
                nc.vector.reciprocal(qr[:], qnorm[:, qt:qt + 1])
                nc.scalar.activation(nqf[:, qt % 3, :], qf[:], AF.Copy,
                                     scale=qr[:])
                nqb = wk.tile([128, D], BF16, tag="nqb")
                nc.vector.tensor_copy(nqb[:], nqf[:, qt % 3, :])
                pq = psQ.tile([128, 256], BF16, tag="pq")
                for h in range(2):
                    nc.tensor.transpose(pq[:, h * 128:(h + 1) * 128],
                                        nqb[:, h * 128:(h + 1) * 128],
                                        identB[:])
                nc.vector.tensor_copy(
                    qT[:, qt, :, :],
                    pq[:].rearrange("p (h j) -> p h j", h=2))

                # -- matmul + pool + window screen --
                cv = wk.tile([128, NCAND], F16, tag="cv")
                ci = wk.tile([128, NCAND], U16, tag="ci")
                for w in range(NW):
                    if w == 4 and mid_cb is not None:
                        mid_cb()
                    pl = pool2.tile([128, WIN], F16, tag="pl")
                    ps0 = psS.tile([128, SUP], F32, tag="ps")
                    ps1 = psS.tile([128, SUP], F32, tag="ps")
                    pst = [ps0, ps1]
                    # h-outer over a super pair: one stationary load
                    # covers 4 consecutive matmuls (deduped in BIR).
                    for h in range(2):
                        for si in range(2):
                            s = w * 2 + si
                            for b2 in range(2):
                                nc.tensor.matmul(
                                    pst[si][:, b2 * 512:(b2 + 1) * 512],
                                    qT[:, qt, h, :],
                                    bankT[:, h, s * SUP + b2 * 512:
                                          s * SUP + (b2 + 1) * 512],
                                    start=(h == 0), stop=(h == 1))
                    # DVE TT may read at most one PSUM operand: scalar
                    # evacuates the odd elements, DVE maxes vs the evens.
                    for si in range(2):
                        psv = pst[si][:].rearrange(
                            "p (m two) -> p two m", two=2)
                        hb = wk.tile([128, PW], F16, tag="hb")
                        nc.scalar.activation(hb[:], psv[:, 1, :], AF.Copy)
                        nc.vector.tensor_tensor(
                            out=pl[:, si * PW:(si + 1) * PW],
                            in0=psv[:, 0, :], in1=hb[:], op=AL.max)
                    nc.vector.max(out=cv[:, w * 8:(w + 1) * 8], in_=pl[:])
                    nc.vector.max_index(
                        out=ci[:, w * 8:(w + 1) * 8],
                        in_max=cv[:, w * 8:(w + 1) * 8], in_values=pl[:])

                # -- prune to LP: pack (value*0.5 + local_idx*2^-26) --
                cif = wk.tile([128, NCAND], F32, tag="cif")
                nc.vector.tensor_copy(cif[:], ci[:])
                packed = wk.tile([128, NCAND], F32, tag="packed")
                nc.vector.tensor_scalar(out=packed[:], in0=cif[:],
                                        scalar1=EPACK, scalar2=None,
                                        op0=AL.mult)
                hcv = wk.tile([128, NCAND], F32, tag="hcv")
                nc.vector.tensor_scalar(out=hcv[:], in0=cv[:], scalar1=0.5,
                                        scalar2=None, op0=AL.mult)
                nc.vector.tensor_tensor(out=packed[:], in0=packed[:],
                                        in1=hcv[:], op=AL.add)
                pk8 = wk.tile([128, 8], F32, tag="pk8")
                nc.vector.max(out=pk8[:], in_=packed[:])
                pB = wk.tile([128, NCAND], F32, tag="pB")
                nc.vector.match_replace(out=pB[:], in_to_replace=pk8[:],
                                        in_values=packed[:], imm_value=NEG)
                pk2 = wk.tile([128, 8], F32, tag="pk2")
                nc.vector.max(out=pk2[:], in_=pB[:])
                pos8 = wk.tile([128, 8], U16, tag="pos8")
                nc.vector.max_index(out=pos8[:], in_max=pk8[:],
                                    in_values=packed[:])
                pos2 = wk.tile([128, 8], U16, tag="pos2")
                nc.vector.max_index(out=pos2[:], in_max=pk2[:],
                                    in_values=pB[:])
                pk = wk.tile([128, LP], F32, tag="pk")
                nc.vector.tensor_copy(pk[:, 0:8], pk8[:])
                nc.vector.tensor_copy(pk[:, 8:LP], pk2[:, 0:LP - 8])
                pos = wk.tile([128, LP], U16, tag="pos")
                nc.vector.tensor_copy(pos[:, 0:8], pos8[:])
                nc.vector.tensor_copy(pos[:, 8:LP], pos2[:, 0:LP - 8])
                # wv = pos >> 3 gives the window id (8 candidates per window)
                # window id from candidate position; local idx by unpacking
                wv = wk.tile([128, LP], U16, tag="wv")
                nc.vector.tensor_scalar(out=wv[:], in0=pos[:],
                                        scalar1=3, scalar2=None,
                                        op0=AL.logical_shift_right)
                woff = wk.tile([128, LP], U16, tag="woff")
                nc.vector.tensor_scalar(out=woff[:], in0=wv[:],
                                        scalar1=WIN, scalar2=None,
                                        op0=AL.mult)
                dbl = wk.tile([128, LP], F32, tag="dbl")
                nc.vector.tensor_scalar(out=dbl[:], in0=pk[:], scalar1=2.0,
                                        scalar2=None, op0=AL.mult)
                cvh = wk.tile([128, LP], F16, tag="cvh")
                nc.vector.tensor_copy(cvh[:], dbl[:])
                cvf = wk.tile([128, LP], F32, tag="cvf")
                nc.vector.tensor_copy(cvf[:], cvh[:])
                idr = wk.tile([128, LP], F32, tag="idr")
                nc.vector.tensor_tensor(out=idr[:], in0=dbl[:], in1=cvf[:],
                                        op=AL.subtract)
                lf = wk.tile([128, LP], F32, tag="lf")
                nc.vector.tensor_scalar(out=lf[:], in0=idr[:],
                                        scalar1=float(2 ** 25),
                                        scalar2=None, op0=AL.mult)
                # P = woff + local is the pooled-pair index: the pair is
                # rows (2P, 2P+1) = one [16384, 512] view row
                loc16 = wk.tile([128, LP], U16, tag="loc16")
                nc.vector.tensor_copy(loc16[:], lf[:])
                Pu16 = wk.tile([128, LP], U16, tag="Pu16")
                nc.vector.tensor_tensor(out=Pu16[:], in0=woff[:],
                                        in1=loc16[:], op=AL.add)
                Pu = wk.tile([128, LP], U32, tag="Pu")
                nc.vector.tensor_copy(Pu[:], Pu16[:])
                if DEBUG:
                    cvf_d = wk.tile([128, NCAND], F32, tag="cvf_d")
                    nc.vector.tensor_copy(cvf_d[:], cv[:])
                    nc.sync.dma_start(dbg_cv[qt], cvf_d[:])
                    nc.sync.dma_start(dbg_ci[qt], ci[:])
                    nc.sync.dma_start(dbg_rows[qt], Pu[:])

                # -- gather LP row-pairs (2KB contiguous descriptors) --
                G = gat.tile([128, LP, 2 * D], F32, tag="G")
                for j in range(LP):
                    nc.gpsimd.indirect_dma_start(
                        out=G[:, j, :], out_offset=None, in_=mem_pairs,
                        in_offset=IndirectOffsetOnAxis(ap=Pu[:, j:j + 1],
                                                       axis=0))
                return (G,)

            def refine(qt, G):
                """exact re-dot, top-8 softmax weights, PE weighted sum."""
                dots = wk.tile([128, 2 * LP], F32, tag="dots")
                scr = scrp.tile([128, D], F32, tag="scr")
                gn2 = wk.tile([128, 2 * LP], F32, tag="gn2")
                gsq = scrp.tile([128, D], F32, tag="gsq")
                for j in range(LP):
                    for h in range(2):
                        nc.vector.scalar_tensor_tensor(
                            out=scr[:],
                            in0=G[:, j, h * D:(h + 1) * D],
                            scalar=1.0, in1=nqf[:, qt % 3, :],
                            op0=AL.mult, op1=AL.mult,
                            accum_out=dots[:, 2 * j + h:2 * j + h + 1])
                        nc.scalar.activation(
                            gsq[:], G[:, j, h * D:(h + 1) * D], AF.Square,
                            accum_out=gn2[:, 2 * j + h:2 * j + h + 1])
                gn = wk.tile([128, 2 * LP], F32, tag="gn")
                nc.scalar.activation(gn[:], gn2[:], AF.Sqrt)
                rnq = wk.tile([128, 2 * LP], F32, tag="rnq")
                nc.vector.reciprocal(rnq[:], gn[:])
                cos = wk.tile([128, 2 * LP], F32, tag="cos")
                nc.vector.tensor_tensor(out=cos[:], in0=dots[:], in1=rnq[:],
                                        op=AL.mult)
                msk = wk.tile([128, 2 * LP], F32, tag="msk")
                nc.vector.tensor_scalar(out=msk[:], in0=cos[:],
                                        scalar1=SELF_MATCH, scalar2=NEG,
                                        op0=AL.is_ge, op1=AL.mult)
                nc.vector.tensor_tensor(out=cos[:], in0=cos[:], in1=msk[:],
                                        op=AL.add)
                t8 = wk.tile([128, 8], F32, tag="t8")
                nc.vector.max(out=t8[:], in_=cos[:])
                sh = wk.tile([128, 2 * LP], F32, tag="sh")
                nc.vector.tensor_scalar(out=sh[:], in0=cos[:],
                                        scalar1=t8[:, 0:1], scalar2=None,
                                        op0=AL.subtract)
                wm = wk.tile([128, 2 * LP], F32, tag="wm")
                nc.vector.tensor_scalar(out=wm[:], in0=cos[:],
                                        scalar1=t8[:, 7:8], scalar2=None,
                                        op0=AL.is_ge)
                ex = wk.tile([128, 2 * LP], F32, tag="ex")
                nc.scalar.activation(ex[:], sh[:], AF.Exp)
                wts = wk.tile([128, 2 * LP], F32, tag="wts")
                nc.vector.tensor_tensor(out=wts[:], in0=ex[:], in1=wm[:],
                                        op=AL.mult)
                if DEBUG:
                    nc.sync.dma_start(dbg_cos[qt], cos[:])
                    nc.sync.dma_start(dbg_wts[qt], wts[:])

                acc = psA.tile([128, D], F32, tag="acc")
                for jj in range(2 * LP):
                    dg = dgp.tile([128, 128], F32, tag="dg")
                    nc.scalar.activation(dg[:], identF[:], AF.Copy,
                                         scale=wts[:, jj:jj + 1])
                    nc.tensor.matmul(
                        acc[:], dg[:],
                        G[:, jj // 2, (jj % 2) * D:(jj % 2 + 1) * D],
                        start=(jj == 0), stop=(jj == 2 * LP - 1))
                accF = wk.tile([128, D], F32, tag="accF")
                nc.scalar.activation(accF[:], acc[:], AF.Copy)
                an2 = wk.tile([128, 1], F32, tag="an2")
                scrN = scrp.tile([128, D], F32, tag="scrN")
                nc.vector.scalar_tensor_tensor(
                    out=scrN[:], in0=accF[:], scalar=1.0, in1=accF[:],
                    op0=AL.mult, op1=AL.mult, accum_out=an2[:])
                an = wk.tile([128, 1], F32, tag="an")
                nc.scalar.activation(an[:], an2[:], AF.Sqrt)
                ar = wk.tile([128, 1], F32, tag="ar")
                nc.vector.reciprocal(ar[:], an[:])
                scl = wk.tile([128, 1], F32, tag="scl")
                nc.vector.tensor_tensor(out=scl[:], in0=ar[:],
                                        in1=qnorm[:, qt:qt + 1], op=AL.mult)
                ot = wk.tile([128, D], F32, tag="ot")
                nc.scalar.activation(ot[:], accF[:], AF.Copy, scale=scl[:])
                nc.sync.dma_start(out[qt * 128:(qt + 1) * 128, :], ot[:])

            prev = None
            for qt in range(NQT):
                cb = (lambda q=qt, p=prev: refine(q - 1, *p)) if prev else None
                prev = screen(qt, mid_cb=cb)
            refine(NQT - 1, *prev)

    return nc


_CACHED_NC = None


def _get_nc():
    global _CACHED_NC
    if _CACHED_NC is None:
        _install_patches()
        _CACHED_NC = _build()
    return _CACHED_NC


def kernel(query, memory, k):
    query = np.ascontiguousarray(np.asarray(query, dtype=np.float32))
    memory = np.ascontiguousarray(np.asarray(memory, dtype=np.float32))
    k_val = int(np.asarray(k))
    assert query.shape == (B, D) and memory.shape == (M, D), (query.shape, memory.shape)
    assert k_val == K, f"kernel compiled for k={K}, got {k_val}"

    from concourse.bass_utils import run_bass_kernel_spmd

    nc = _get_nc()
    in_maps = [
        {"q": query[i * B_LOC: (i + 1) * B_LOC], "mem": memory}
        for i in range(N_CORES)
    ]
    res = run_bass_kernel_spmd(nc, in_maps, list(range(N_CORES)))
    return np.concatenate([res.results[i]["out"] for i in range(N_CORES)], axis=0)
